# revision 10
# baseline (speedup 1.0000x reference)
"""Trainium2 Bass kernel for nn_ContextMemory (scatter_memory).

Math (B=8, S=2048, M=512, E=1024, H=16, D=64):
  read:  mem_out = LN(MHA(query, mem, mem) + query)         [B,S,E]
  write: softmax over a single key is identically 1, so the write-MHA
         collapses to   upd = (mean_s(query) @ Wv_w.T + bv_w) @ Wo_w.T + bo_w
         broadcast over memory slots.
         new_mem = LN(gate*upd + (1-gate)*mem),  gate = sigmoid([mem,upd]@Gw.T+gb)

Distribution: pure data-parallel over batch, 1 batch element per NeuronCore,
weights replicated, zero collectives.

On-device layouts (per core):
  xT   = query^T   [E(8x128), S]  bf16   (DMA-transposed load)
  qT   = (Wq query^T + bq)        bf16   [E, S]
  kT   = (Wk mem^T + bk)          bf16   [E, M]
  v    = (mem Wv^T + bv)          bf16   [M(4x128), E]
  scores^T per head pair via row-tiled K=64 matmuls (2 concurrent),
  exp on ACT (scale=1/8, no max-subtraction: |scores/8| < 3),
  softmax sums via ones-matmuls, partition-broadcast via DRAM bounce,
  attn^T = (exp^T @ v) / sums via col-tiled M=64 matmuls (2 concurrent),
  out-proj back to [S, E] rows, +bias +residual, LayerNorm via bn_stats
  (rstd = exp(-0.5 ln(var+eps)) so ACT only ever needs the exp/ln table set).
"""

import sys
import numpy as np
from contextlib import ExitStack

if "/opt/trn_rl_repo" not in sys.path:
    sys.path.insert(0, "/opt/trn_rl_repo")

import concourse.bass as bass
import concourse.tile as tile
from concourse import bacc, mybir

B, S, M, E, H = 8, 2048, 512, 1024, 16
D = E // H
EPS = 1e-5
N_CORES = 8
F32 = mybir.dt.float32
BF16 = mybir.dt.bfloat16

QT = 4            # S tiles of 512 ("qtiles")
QTS = S // QT     # 512
QC = QTS // 128   # q chunks of 128 per qtile
KC = M // 128     # 4 key chunks
ET = E // 128     # 8 embedding tiles
HP = H // 2       # 8 head pairs


def _bcast_ap(dram_ap, parts=128):
    """DRAM AP broadcast along a new partition dim (stride 0)."""
    return bass.AP(
        tensor=dram_ap.tensor,
        offset=dram_ap.offset,
        ap=[[0, parts]] + list(dram_ap.ap),
    )


def _build_program():
    nc = bacc.Bacc(
        "TRN2", target_bir_lowering=False, debug=False, num_devices=N_CORES
    )

    def din(name, shape, dt):
        return nc.dram_tensor(name, shape, dt, kind="ExternalInput").ap()

    def dout(name, shape, dt):
        return nc.dram_tensor(name, shape, dt, kind="ExternalOutput").ap()

    t = {}
    t["q_f32"] = din("q_f32", [S, E], F32)
    t["q_bf"] = din("q_bf", [S, E], BF16)
    t["mem_f32"] = din("mem_f32", [M, E], F32)
    t["mem_bf"] = din("mem_bf", [M, E], BF16)
    for w in ["wq_t", "wk_t", "wv_t", "wo_t", "gm_t", "gu_t", "wvw_t", "wow_t"]:
        t[w] = din(w, [E, E], BF16)
    for v in ["bq", "bk", "bv", "rob", "rng", "rnb", "wng", "wnb", "gb",
              "bvw", "bow"]:
        t[v] = din(v, [E], F32)
    t["mem_out"] = dout("mem_out", [S, E], F32)
    t["new_mem"] = dout("new_mem", [M, E], F32)

    with tile.TileContext(nc) as tc:
        _emit(nc, tc, t)
    nc.compile()
    return nc


def _emit(nc, tc, t):
    AF = mybir.ActivationFunctionType
    ALU = mybir.AluOpType
    AX = mybir.AxisListType

    with ExitStack() as ctx:
        # ---------- long-lived pools ----------
        singles = ctx.enter_context(tc.tile_pool(name="singles", bufs=1))
        consts = ctx.enter_context(tc.tile_pool(name="consts", bufs=1))
        stat_pool = ctx.enter_context(tc.tile_pool(name="stat", bufs=4))
        mm2 = ctx.enter_context(tc.tile_pool(name="mm2", bufs=3, space="PSUM"))
        av_ps_pool = ctx.enter_context(
            tc.tile_pool(name="avps", bufs=1, space="PSUM"))
        sums_ps_pool = ctx.enter_context(
            tc.tile_pool(name="sumsps", bufs=1, space="PSUM"))
        dram_pool = ctx.enter_context(
            tc.tile_pool(name="dscr", bufs=1, space="DRAM"))

        # ---------- constants ----------
        def load_row(dram_ap):
            r = consts.tile([128, E], F32, tag=dram_ap.tensor.name + "_row")
            nc.sync.dma_start(out=r, in_=_bcast_ap(dram_ap))
            return r

        bv_row = load_row(t["bv"])
        rob_row = load_row(t["rob"])
        rng_row = load_row(t["rng"])
        rnb_row = load_row(t["rnb"])
        wng_row = load_row(t["wng"])
        wnb_row = load_row(t["wnb"])

        def load_cols(dram_ap):
            c = consts.tile([128, ET], F32, tag=dram_ap.tensor.name + "_col")
            nc.sync.dma_start(out=c, in_=dram_ap.rearrange("(t p) -> p t", p=128))
            return c

        bq_sb = load_cols(t["bq"])
        bk_sb = load_cols(t["bk"])
        bvw_sb = load_cols(t["bvw"])
        bow_sb = load_cols(t["bow"])

        gb_sb = consts.tile([1, E], F32)
        nc.sync.dma_start(out=gb_sb, in_=t["gb"].rearrange("(a e) -> a e", a=1))
        eps_t = consts.tile([128, 1], F32)
        nc.vector.memset(eps_t, EPS)
        ones_bf = consts.tile([128, 1], BF16)
        nc.vector.memset(ones_bf, 1.0)

        # ---------- persistent activation/weight tiles ----------
        mT = singles.tile([128, ET, M], BF16)       # memory^T          8K/part
        kT = singles.tile([128, ET, M], BF16)       # Wk mem^T + bk     8K
        v_sb = singles.tile([128, KC, E], BF16)     # mem Wv^T + bv     8K
        wo_sb = singles.tile([128, ET, E], BF16)    # r_out_w^T         16K
        wq_sb = singles.tile([128, ET, E], BF16)    # wq_t              16K
        wq_part = singles.tile([128, ET, QT], F32)  # query-mean partials

        for et in range(ET):
            nc.sync.dma_start_transpose(
                out=mT[:, et, :], in_=t["mem_bf"][:, et * 128:(et + 1) * 128])
            nc.sync.dma_start(out=wo_sb[:, et, :],
                              in_=t["wo_t"][et * 128:(et + 1) * 128, :])
            nc.sync.dma_start(out=wq_sb[:, et, :],
                              in_=t["wq_t"][et * 128:(et + 1) * 128, :])

        # ---------- phase 1: kT / v projections (scoped weights) ----------
        with tc.tile_pool(name="kvw", bufs=1) as kvw:
            wk_sb = kvw.tile([128, ET, E], BF16)
            wv_sb = kvw.tile([128, ET, E], BF16)
            for ei in range(ET):
                nc.sync.dma_start(out=wk_sb[:, ei, :],
                                  in_=t["wk_t"][ei * 128:(ei + 1) * 128, :])
                nc.sync.dma_start(out=wv_sb[:, ei, :],
                                  in_=t["wv_t"][ei * 128:(ei + 1) * 128, :])

            # kT[eo] = sum_ei wk[ei, eo-cols].T @ mT[ei] -> [128 eo, 512 keys]
            for eo2 in range(ET // 2):
                ps = mm2.tile([128, 2, 512], F32, tag="mm")
                for j in range(2):
                    eo = eo2 * 2 + j
                    for ei in range(ET):
                        nc.tensor.matmul(
                            ps[:, j, :],
                            lhsT=wk_sb[:, ei, eo * 128:(eo + 1) * 128],
                            rhs=mT[:, ei, :],
                            start=(ei == 0), stop=(ei == ET - 1),
                        )
                for j in range(2):
                    eo = eo2 * 2 + j
                    nc.vector.tensor_scalar(
                        out=kT[:, eo, :], in0=ps[:, j, :],
                        scalar1=bk_sb[:, eo:eo + 1], scalar2=None, op0=ALU.add,
                    )

            # v[kc] = mem[kc] @ Wv^T + bv -> [128 keys, 1024]
            for kc in range(KC):
                ps = mm2.tile([128, 2, 512], F32, tag="mm")
                for j in range(2):
                    for ei in range(ET):
                        nc.tensor.matmul(
                            ps[:, j, :],
                            lhsT=mT[:, ei, kc * 128:(kc + 1) * 128],
                            rhs=wv_sb[:, ei, j * 512:(j + 1) * 512],
                            start=(ei == 0), stop=(ei == ET - 1),
                        )
                for j in range(2):
                    nc.vector.tensor_tensor(
                        out=v_sb[:, kc, j * 512:(j + 1) * 512],
                        in0=ps[:, j, :],
                        in1=bv_row[:, j * 512:(j + 1) * 512],
                        op=ALU.add,
                    )

        # ---------- main loop over qtiles ----------


        rscr = dram_pool.tile([QT, H, 512], F32)

        with ExitStack() as qctx:
            xt_pool = qctx.enter_context(tc.tile_pool(name="xt", bufs=2))
            qt_pool = qctx.enter_context(tc.tile_pool(name="qt", bufs=2))
            attn_pool = qctx.enter_context(tc.tile_pool(name="attn", bufs=2))
            exp_pool = qctx.enter_context(tc.tile_pool(name="expp", bufs=3))
            bc_pool = qctx.enter_context(tc.tile_pool(name="bc", bufs=2))
            rc_pool = qctx.enter_context(tc.tile_pool(name="rc", bufs=2))
            ln_pool = qctx.enter_context(tc.tile_pool(name="lnp", bufs=2))
            qres_pool = qctx.enter_context(tc.tile_pool(name="qres", bufs=2))

            for qt in range(QT):
                # --- transposed query load + mean partials ---
                xt = xt_pool.tile([128, ET, QTS], BF16, tag="xt")
                for et in range(ET):
                    nc.sync.dma_start_transpose(
                        out=xt[:, et, :],
                        in_=t["q_bf"][qt * QTS:(qt + 1) * QTS,
                                      et * 128:(et + 1) * 128],
                    )
                for et in range(ET):
                    nc.vector.tensor_reduce(
                        out=wq_part[:, et, qt:qt + 1], in_=xt[:, et, :],
                        axis=AX.X, op=ALU.add,
                    )

                # --- q projection: qT[eo] = sum_ei wq[ei,eo].T @ xT[ei] ---
                qTt = qt_pool.tile([128, ET, QTS], BF16, tag="qT")
                for eo2 in range(ET // 2):
                    ps = mm2.tile([128, 2, 512], F32, tag="mm")
                    for j in range(2):
                        eo = eo2 * 2 + j
                        for ei in range(ET):
                            nc.tensor.matmul(
                                ps[:, j, :],
                                lhsT=wq_sb[:, ei, eo * 128:(eo + 1) * 128],
                                rhs=xt[:, ei, :],
                                start=(ei == 0), stop=(ei == ET - 1),
                            )
                    for j in range(2):
                        eo = eo2 * 2 + j
                        nc.vector.tensor_scalar(
                            out=qTt[:, eo, :], in0=ps[:, j, :],
                            scalar1=bq_sb[:, eo:eo + 1], scalar2=None,
                            op0=ALU.add,
                        )

                # --- attention ---
                attn_qt = attn_pool.tile([128, HP, QTS], BF16, tag="attn")
                for hp in range(HP):
                    h0, h1 = 2 * hp, 2 * hp + 1
                    av_ps = av_ps_pool.tile([128, QTS], F32, tag="av")
                    sums_ps = sums_ps_pool.tile([64, QTS], F32, tag="sums")
                    for kc in range(KC):
                        sc_ps = mm2.tile([128, 2, 512], F32, tag="mm")
                        nc.tensor.matmul(
                            sc_ps[:, 0, :],
                            lhsT=kT[0:64, hp, kc * 128:(kc + 1) * 128],
                            rhs=qTt[0:64, hp, :],
                            start=True, stop=True,
                        )
                        nc.tensor.matmul(
                            sc_ps[:, 1, :],
                            lhsT=kT[64:128, hp, kc * 128:(kc + 1) * 128],
                            rhs=qTt[64:128, hp, :],
                            start=True, stop=True,
                        )
                        ex = exp_pool.tile([128, 2, 512], BF16, tag="exp")
                        nc.scalar.activation(
                            out=ex, in_=sc_ps, func=AF.Exp, scale=0.125)
                        nc.tensor.matmul(
                            sums_ps[0:1, :], lhsT=ones_bf, rhs=ex[:, 0, :],
                            start=(kc == 0), stop=(kc == KC - 1),
                            tile_position=(0, 0), skip_group_check=True,
                        )
                        nc.tensor.matmul(
                            sums_ps[32:33, :], lhsT=ones_bf, rhs=ex[:, 1, :],
                            start=(kc == 0), stop=(kc == KC - 1),
                            tile_position=(0, 32), skip_group_check=True,
                        )
                        nc.tensor.matmul(
                            av_ps[0:64, :],
                            lhsT=v_sb[:, kc, h0 * D:(h0 + 1) * D],
                            rhs=ex[:, 0, :],
                            start=(kc == 0), stop=(kc == KC - 1),
                            tile_position=(0, 0), skip_group_check=True,
                        )
                        nc.tensor.matmul(
                            av_ps[64:128, :],
                            lhsT=v_sb[:, kc, h1 * D:(h1 + 1) * D],
                            rhs=ex[:, 1, :],
                            start=(kc == 0), stop=(kc == KC - 1),
                            tile_position=(0, 64), skip_group_check=True,
                        )
                    rc0 = rc_pool.tile([1, QTS], F32, tag="rcrow0")
                    rc1 = rc_pool.tile([1, QTS], F32, tag="rcrow1")
                    nc.vector.reciprocal(out=rc0, in_=sums_ps[0:1, :])
                    nc.vector.reciprocal(out=rc1, in_=sums_ps[32:33, :])
                    nc.sync.dma_start(
                        out=rscr[qt, h0, :].rearrange("(b a) -> b a", b=1),
                        in_=rc0)
                    nc.sync.dma_start(
                        out=rscr[qt, h1, :].rearrange("(b a) -> b a", b=1),
                        in_=rc1)
                    bc = bc_pool.tile([128, QTS], F32, tag="bc")
                    nc.sync.dma_start(out=bc[0:64, :],
                                      in_=_bcast_ap(rscr[qt, h0, :], 64))
                    nc.sync.dma_start(out=bc[64:128, :],
                                      in_=_bcast_ap(rscr[qt, h1, :], 64))
                    nc.vector.tensor_tensor(
                        out=attn_qt[:, hp, :], in0=av_ps, in1=bc, op=ALU.mult)

                # --- out-proj + residual + LayerNorm ---
                for c in range(QC):
                    mo_ps = mm2.tile([128, 2, 512], F32, tag="mm")
                    for e2 in range(2):
                        for hp in range(HP):
                            nc.tensor.matmul(
                                mo_ps[:, e2, :],
                                lhsT=attn_qt[:, hp, c * 128:(c + 1) * 128],
                                rhs=wo_sb[:, hp, e2 * 512:(e2 + 1) * 512],
                                start=(hp == 0), stop=(hp == HP - 1),
                            )
                    qres = qres_pool.tile([128, E], F32, tag="qres")
                    row0 = qt * QTS + c * 128
                    nc.sync.dma_start(out=qres,
                                      in_=t["q_f32"][row0:row0 + 128, :])
                    x_sb = ln_pool.tile([128, E], F32, tag="x")
                    for e2 in range(2):
                        nc.vector.tensor_tensor(
                            out=x_sb[:, e2 * 512:(e2 + 1) * 512],
                            in0=mo_ps[:, e2, :],
                            in1=rob_row[:, e2 * 512:(e2 + 1) * 512],
                            op=ALU.add,
                        )
                    nc.vector.tensor_tensor(out=x_sb, in0=x_sb, in1=qres,
                                            op=ALU.add)
                    _layernorm_store(
                        nc, stat_pool, ln_pool, x_sb, rng_row, rnb_row, eps_t,
                        t["mem_out"][row0:row0 + 128, :], AF, ALU,
                    )

        # ---------- write phase ----------
        with tc.tile_pool(name="wwr", bufs=1) as wwr, \
                tc.tile_pool(name="wrp", bufs=1) as wr_pool:
            wq_sum = wwr.tile([128, ET], F32)
            nc.vector.tensor_reduce(out=wq_sum, in_=wq_part, axis=AX.X,
                                    op=ALU.add)
            wq_bf = wwr.tile([128, ET], BF16)
            nc.vector.tensor_scalar(
                out=wq_bf, in0=wq_sum, scalar1=1.0 / S, scalar2=None,
                op0=ALU.mult)

            wvw_sb = wwr.tile([128, ET, E], BF16)
            wow_sb = wwr.tile([128, ET, E], BF16)
            gu_sb = wwr.tile([128, ET, E], BF16)
            gm_sb = wwr.tile([128, ET, E], BF16)
            for ei in range(ET):
                nc.sync.dma_start(out=wvw_sb[:, ei, :],
                                  in_=t["wvw_t"][ei * 128:(ei + 1) * 128, :])
                nc.sync.dma_start(out=wow_sb[:, ei, :],
                                  in_=t["wow_t"][ei * 128:(ei + 1) * 128, :])
                nc.sync.dma_start(out=gu_sb[:, ei, :],
                                  in_=t["gu_t"][ei * 128:(ei + 1) * 128, :])
                nc.sync.dma_start(out=gm_sb[:, ei, :],
                                  in_=t["gm_t"][ei * 128:(ei + 1) * 128, :])

            # vw^T = Wv_w wq + bvw ; upd^T = Wo_w vw + bow   [E(8x128), 1]
            vw_bf = wwr.tile([128, ET], BF16)
            upd_f32 = wwr.tile([128, ET], F32)
            upd_bf = wwr.tile([128, ET], BF16)
            for eo in range(ET):
                ps = mm2.tile([128, 2, 512], F32, tag="mm")
                for ei in range(ET):
                    nc.tensor.matmul(
                        ps[0:128, 0, 0:1],
                        lhsT=wvw_sb[:, ei, eo * 128:(eo + 1) * 128],
                        rhs=wq_bf[:, ei:ei + 1],
                        start=(ei == 0), stop=(ei == ET - 1),
                    )
                nc.vector.tensor_scalar(
                    out=vw_bf[:, eo:eo + 1], in0=ps[0:128, 0, 0:1],
                    scalar1=bvw_sb[:, eo:eo + 1], scalar2=None, op0=ALU.add)
            for eo in range(ET):
                ps = mm2.tile([128, 2, 512], F32, tag="mm")
                for ei in range(ET):
                    nc.tensor.matmul(
                        ps[0:128, 0, 0:1],
                        lhsT=wow_sb[:, ei, eo * 128:(eo + 1) * 128],
                        rhs=vw_bf[:, ei:ei + 1],
                        start=(ei == 0), stop=(ei == ET - 1),
                    )
                nc.vector.tensor_scalar(
                    out=upd_f32[:, eo:eo + 1], in0=ps[0:128, 0, 0:1],
                    scalar1=bow_sb[:, eo:eo + 1], scalar2=None, op0=ALU.add)
            nc.vector.tensor_scalar(
                out=upd_bf, in0=upd_f32, scalar1=0.0, scalar2=None, op0=ALU.add)

            # ug_row = upd @ Gu^T + gate_b   [1, E]
            ug_ps = mm2.tile([128, 2, 512], F32, tag="mm")
            for e2 in range(2):
                for ei in range(ET):
                    nc.tensor.matmul(
                        ug_ps[0:1, e2, :],
                        lhsT=upd_bf[:, ei:ei + 1],
                        rhs=gu_sb[:, ei, e2 * 512:(e2 + 1) * 512],
                        start=(ei == 0), stop=(ei == ET - 1),
                    )
            ug_sb = wwr.tile([1, E], F32)
            nc.vector.tensor_tensor(
                out=ug_sb, in0=ug_ps[0:1, :, :].rearrange("p a b -> p (a b)"),
                in1=gb_sb, op=ALU.add)

            # bounce upd + ug rows through DRAM for partition-broadcast
            row_scr = dram_pool.tile([2, E], F32)
            nc.sync.dma_start(
                out=row_scr[0, :].rearrange("(t p) -> p t", p=128), in_=upd_f32)
            nc.sync.dma_start(out=row_scr[1, :].rearrange("(a e) -> a e", a=1), in_=ug_sb)
            ubc = wwr.tile([128, 2, E], F32)
            nc.sync.dma_start(out=ubc, in_=_bcast_ap(row_scr))

            # gate + combine + LN per key chunk
            for kc in range(KC):
                gp_ps = mm2.tile([128, 2, 512], F32, tag="mm")
                for e2 in range(2):
                    for ei in range(ET):
                        nc.tensor.matmul(
                            gp_ps[:, e2, :],
                            lhsT=mT[:, ei, kc * 128:(kc + 1) * 128],
                            rhs=gm_sb[:, ei, e2 * 512:(e2 + 1) * 512],
                            start=(ei == 0), stop=(ei == ET - 1),
                        )
                z = wr_pool.tile([128, E], F32, tag="z")
                for e2 in range(2):
                    nc.vector.tensor_tensor(
                        out=z[:, e2 * 512:(e2 + 1) * 512],
                        in0=gp_ps[:, e2, :],
                        in1=ubc[:, 1, e2 * 512:(e2 + 1) * 512],
                        op=ALU.add)
                eneg = wr_pool.tile([128, E], F32, tag="eneg")
                nc.scalar.activation(out=eneg, in_=z, func=AF.Exp, scale=-1.0)
                nc.vector.tensor_scalar(
                    out=eneg, in0=eneg, scalar1=1.0, scalar2=None, op0=ALU.add)
                gate = wr_pool.tile([128, E], F32, tag="gate")
                nc.vector.reciprocal(out=gate, in_=eneg)
                msb = wr_pool.tile([128, E], F32, tag="msb")
                nc.sync.dma_start(
                    out=msb, in_=t["mem_f32"][kc * 128:(kc + 1) * 128, :])
                dd = wr_pool.tile([128, E], F32, tag="dd")
                nc.vector.tensor_tensor(out=dd, in0=ubc[:, 0, :], in1=msb,
                                        op=ALU.subtract)
                nc.vector.tensor_tensor(out=dd, in0=gate, in1=dd, op=ALU.mult)
                nm_in = wr_pool.tile([128, E], F32, tag="nmin")
                nc.vector.tensor_tensor(out=nm_in, in0=dd, in1=msb, op=ALU.add)
                _layernorm_store(
                    nc, stat_pool, wr_pool, nm_in, wng_row, wnb_row, eps_t,
                    t["new_mem"][kc * 128:(kc + 1) * 128, :], AF, ALU,
                )


def _layernorm_store(nc, stat_pool, out_pool, x_sb, g_row, b_row, eps_t,
                     dram_out, AF, ALU):
    """LayerNorm x_sb [128, E] over free dim, scale/shift, DMA to dram_out."""
    stats = stat_pool.tile([128, 2, 6], F32, tag="bnst")
    for sg in range(2):
        nc.vector.bn_stats(out=stats[:, sg, :],
                           in_=x_sb[:, sg * 512:(sg + 1) * 512])
    mv = stat_pool.tile([128, 2], F32, tag="bnmv")
    nc.vector.bn_aggr(out=mv, in_=stats)
    lnv = stat_pool.tile([128, 1], F32, tag="lnv")
    nc.scalar.activation(out=lnv, in_=mv[:, 1:2], func=AF.Ln, bias=eps_t,
                         scale=1.0)
    rstd = stat_pool.tile([128, 1], F32, tag="rstd")
    nc.scalar.activation(out=rstd, in_=lnv, func=AF.Exp, scale=-0.5)
    y = out_pool.tile([128, E], F32, tag="y")
    nc.vector.tensor_scalar(
        out=y, in0=x_sb, scalar1=mv[:, 0:1], scalar2=rstd,
        op0=ALU.subtract, op1=ALU.mult)
    nc.vector.tensor_tensor(out=y, in0=y, in1=g_row, op=ALU.mult)
    nc.vector.tensor_tensor(out=y, in0=y, in1=b_row, op=ALU.add)
    nc.sync.dma_start(out=dram_out, in_=y)


_CACHE = {}


def _get_program():
    if "nc" not in _CACHE:
        _CACHE["nc"] = _build_program()
    return _CACHE["nc"]


def _prep_in_maps(inputs):
    import ml_dtypes

    bf16 = ml_dtypes.bfloat16
    q = np.ascontiguousarray(np.asarray(inputs["query"], np.float32))
    mem = np.ascontiguousarray(np.asarray(inputs["memory"], np.float32)[0])
    r_in_w = np.asarray(inputs["r_in_w"], np.float32)
    r_in_b = np.asarray(inputs["r_in_b"], np.float32)
    w_in_w = np.asarray(inputs["w_in_w"], np.float32)
    w_in_b = np.asarray(inputs["w_in_b"], np.float32)

    def bf(x):
        return np.ascontiguousarray(np.asarray(x).astype(bf16))

    shared = {
        "mem_f32": mem,
        "mem_bf": bf(mem),
        "wq_t": bf(r_in_w[:E].T),
        "wk_t": bf(r_in_w[E:2 * E].T),
        "wv_t": bf(r_in_w[2 * E:].T),
        "wo_t": bf(np.asarray(inputs["r_out_w"], np.float32).T),
        "gm_t": bf(np.asarray(inputs["gate_w"], np.float32)[:, :E].T),
        "gu_t": bf(np.asarray(inputs["gate_w"], np.float32)[:, E:].T),
        "wvw_t": bf(w_in_w[2 * E:3 * E].T),
        "wow_t": bf(np.asarray(inputs["w_out_w"], np.float32).T),
        "bq": np.ascontiguousarray(r_in_b[:E]),
        "bk": np.ascontiguousarray(r_in_b[E:2 * E]),
        "bv": np.ascontiguousarray(r_in_b[2 * E:]),
        "rob": np.asarray(inputs["r_out_b"], np.float32),
        "rng": np.asarray(inputs["rn_g"], np.float32),
        "rnb": np.asarray(inputs["rn_b"], np.float32),
        "wng": np.asarray(inputs["wn_g"], np.float32),
        "wnb": np.asarray(inputs["wn_b"], np.float32),
        "gb": np.asarray(inputs["gate_b"], np.float32),
        "bvw": np.ascontiguousarray(w_in_b[2 * E:3 * E]),
        "bow": np.asarray(inputs["w_out_b"], np.float32),
    }
    in_maps = []
    for b in range(B):
        m = dict(shared)
        m["q_f32"] = np.ascontiguousarray(q[b])
        m["q_bf"] = bf(q[b])
        in_maps.append(m)
    return in_maps


def kernel(**inputs):
    from concourse.bass_utils import run_bass_kernel_spmd

    nc = _get_program()
    in_maps = _prep_in_maps(inputs)
    res = run_bass_kernel_spmd(nc, in_maps, core_ids=list(range(N_CORES)))
    mem_out = np.stack([res.results[b]["mem_out"] for b in range(B)])
    new_mem = np.stack([res.results[b]["new_mem"] for b in range(B)])
    return mem_out.astype(np.float32), new_mem.astype(np.float32)
